# revision 79
# baseline (speedup 1.0000x reference)
"""Trainium2 Bass kernel for nn_L1RegressionMoEActionHead.

Data-parallel over batch: 16 batch elements -> 2 per core x 8 cores.
Only the selected expert's weights are shipped (host-sliced); scale factors
(1/sqrt(HD), sigmoid(gating)) and LayerNorm gamma/beta are folded into the
weights on the host.

All six Q/K/V projections and the o-projection run in fp8(e4m3)
DoubleRow mode (2x PE throughput, 256-deep contraction per matmul);
the attention value/denominator matmuls are DoubleRow too (fp8 p-tiles
and V in key-pair layout).  Quantization scales are powers of two folded
host-side: q/k descale rides the softmax exp() activation scale, v
descale is folded into W_o, o-proj descale rides the residual add.
Projections and attention are interleaved per head so the PE (matmuls),
ACT (psum moves + exp), DVE and GpSimd (rope combine, normalization)
all stay busy; K heads are projected first (their loads arrive first),
per-head q tiles rotate in a 2-deep pool.  RoPE's rotate-half is a DVE
stream_shuffle with the sign folded into the sin tables.  The softmax
denominator comes from a ones-stationary DoubleRow matmul that
broadcasts the key-sum to every partition (no reciprocal broadcast
needed).  o-proj and V biases are folded into the residual input /
rank-1 matmuls.  Phase-C weights and residuals prefetch during
attention; Phase C software-pipelines o-proj ahead of FFN.
"""

import math

import numpy as np
import ml_dtypes

B = 16
T = 512
KA = 256
KT = 256
DIM = 1024
NH = 8
HD = 128
E = 8
EPS = 1e-5

NCORES = 8
BLOC = B // NCORES          # 2 batch elements per core
TOKQ = BLOC * T             # 1024 query tokens per core
TOKK = BLOC * KA            # 512 kv tokens per core (each of h_a / h_t)
NCT = DIM // 128            # 8 contraction tiles
NPAIR = NCT // 2            # 4 DoubleRow pairs

BF16 = ml_dtypes.bfloat16
FP8 = ml_dtypes.float8_e4m3

# power-of-two quantization scales (folded host-side, exact)
QS_A = 64.0      # on W_qa*sc     (std 0.00177 -> 0.113)
QS_T = 128.0     # on W_qt*sc*g   (std 0.00088 -> 0.113)
KS = 8.0         # on W_ka / W_kt (std 0.02  -> 0.16)
VS = 8.0         # on W_va / W_vt (+bias); 1/VS into W_o
OS = 64.0        # on W_o/VS (fp8 o-proj); descale in the residual add

_CACHE = {}


def _rope_cos_sin(L):
    inv_freq = 1.0 / (10000.0 ** (np.arange(0, HD, 2, dtype=np.float32) / HD))
    freqs = np.arange(L, dtype=np.float32)[:, None] * inv_freq[None, :]
    emb = np.concatenate([freqs, freqs], axis=-1)   # (L, HD)
    return np.cos(emb), np.sin(emb)


def _rhat():
    # unsigned pair-swap permutation; the rotate_half sign lives in the
    # (sign-folded) sin tables
    R = np.zeros((HD, HD), dtype=np.float32)
    idx = np.arange(0, HD, 2)
    R[idx, idx + 1] = 1.0
    R[idx + 1, idx] = 1.0
    return R


def build_program():
    import concourse.bass as bass
    import concourse.mybir as mybir
    import concourse.tile as tile
    from concourse import bacc
    from contextlib import ExitStack

    f32 = mybir.dt.float32
    bf16 = mybir.dt.bfloat16
    fp8 = mybir.dt.float8e4
    AF = mybir.ActivationFunctionType
    ALU = mybir.AluOpType
    DR = mybir.MatmulPerfMode.DoubleRow

    nc = bacc.Bacc("TRN2", target_bir_lowering=False, debug=False)

    # ---------------- DRAM parameters ----------------
    def din(name, shape, dt):
        return nc.dram_tensor(name, list(shape), dt, kind="ExternalInput")

    x8 = din("x8", (DIM, TOKQ), fp8)
    ha8 = din("ha8", (DIM, TOKK), fp8)
    ht8 = din("ht8", (DIM, TOKK), fp8)
    xnat = din("xnat", (TOKQ, DIM), bf16)     # x + b_o (residual, host-folded)

    wqa8 = din("wqa8", (DIM, DIM), fp8)
    wqt8 = din("wqt8", (DIM, DIM), fp8)
    wka8 = din("wka8", (DIM, DIM), fp8)
    wkt8 = din("wkt8", (DIM, DIM), fp8)
    wva8 = din("wva8", (DIM, DIM), fp8)
    wvt8 = din("wvt8", (DIM, DIM), fp8)
    wo8 = din("wo8", (DIM, DIM), fp8)
    wfT = din("wfT", (DIM, DIM), bf16)

    biascols = din("biascols", (128, 4 * NH), f32)   # bqa|bqt|bka|bkt (scaled)
    bva_r = din("bva_r", (1, DIM), bf16)             # x VS
    bvt_r = din("bvt_r", (1, DIM), bf16)             # x VS
    bf_row = din("bf_row", (1, DIM), bf16)

    out_d = nc.dram_tensor("out", [TOKQ, DIM], f32, kind="ExternalOutput")

    # ---------------- inline constants ----------------
    cos_q, sin_q = _rope_cos_sin(T)         # (T, HD)
    cos_k, sin_k = _rope_cos_sin(KA)        # (KA, HD)
    # rot(q) = pair-swap(q) with even lanes negated; the stream_shuffle does
    # the swap and the sign is folded into the sin tables (even rows -1).
    sgn = np.where(np.arange(HD) % 2 == 0, -1.0, 1.0)[:, None].astype(np.float32)
    cosqT = np.ascontiguousarray(cos_q.T).astype(BF16)          # (HD, T)
    sinqT = np.ascontiguousarray(sin_q.T * sgn).astype(BF16)
    coskT = np.ascontiguousarray(np.tile(cos_k.T, (1, BLOC))).astype(BF16)  # (HD, TOKK)
    sinkT = np.ascontiguousarray(np.tile(sin_k.T * sgn, (1, BLOC))).astype(BF16)
    PAIRSWAP = [i ^ 1 for i in range(32)]

    # pack all bf16 constants into one blob: cols =
    # cosq[0:512] sinq[512:1024] cosk[1024:1536] sink[1536:2048]
    # rhatT[2048:2176] ident[2176:2304] ones[2304:2432]
    blob_bf = np.concatenate([
        cosqT, sinqT, coskT, sinkT,
        np.ascontiguousarray(_rhat().T).astype(BF16),
        np.eye(128, dtype=np.float32).astype(BF16),
        np.ones((128, 128), dtype=np.float32).astype(BF16),
    ], axis=1)
    c_blob_bf = nc.inline_tensor(np.ascontiguousarray(blob_bf), "c_blob_bf")
    # f32 blob: eps[0:1]
    blob_f = np.concatenate([
        np.full((128, 1), EPS, dtype=np.float32),
        np.ones((128, 1), dtype=np.float32),
    ], axis=1)
    c_blob_f = nc.inline_tensor(np.ascontiguousarray(blob_f), "c_blob_f")
    # fp8 ones, 256-deep (pair layout) for the DoubleRow denominator matmul
    c_ones8 = nc.inline_tensor(
        np.ones((128, 256), dtype=np.float32).astype(FP8), "c_ones8")

    # exp() descale: scores carry the q/k quantization scales
    ESC_A = 1.0 / (QS_A * KS)
    ESC_T = 1.0 / (QS_T * KS)
    NTILES = TOKQ // 128

    with tile.TileContext(nc) as tc, ExitStack() as ctx:
        persist = ctx.enter_context(tc.tile_pool(name="persist", bufs=1))
        consts = ctx.enter_context(tc.tile_pool(name="consts", bufs=1))

        def cload(dram, shape, dt, tag):
            # consts go out on the ACT engine's DGE so the SP queue leads
            # with the activation/weight loads the first matmuls need.
            t = consts.tile(list(shape), dt, name=tag, tag=tag)
            nc.scalar.dma_start(t[:], dram.ap())
            return t

        sb_cb = cload(c_blob_bf, (128, blob_bf.shape[1]), bf16, "cb")
        sb_cf = cload(c_blob_f, (128, blob_f.shape[1]), f32, "cf")
        sb_bias = cload(biascols, (128, 4 * NH), f32, "biasc")
        sb_cosq = sb_cb[:, 0:512]
        sb_sinq = sb_cb[:, 512:1024]
        sb_cosk = sb_cb[:, 1024:1536]
        sb_sink = sb_cb[:, 1536:2048]
        sb_rhatT = sb_cb[:, 2048:2176]
        sb_ident = sb_cb[:, 2176:2304]
        sb_ones_row = sb_cb[0:1, 2304:2432]
        sb_ones128 = sb_cb[:, 2304:2432]
        sb_eps = sb_cf[:, 0:1]
        sb_bcol = {"qa": sb_bias[:, 0:NH], "qt": sb_bias[:, NH:2 * NH],
                   "ka": sb_bias[:, 2 * NH:3 * NH], "kt": sb_bias[:, 3 * NH:4 * NH]}
        sb_bf = cload(bf_row, (1, DIM), bf16, "bf")
        sb_ones8 = cload(c_ones8, (128, 2, 128), fp8, "ones8")

        # phase-C weight tiles (DMAs queue behind the A/B loads)
        wot = persist.tile([128, NCT, DIM], fp8, name="wot", tag="wo")
        wft = persist.tile([128, NCT, DIM], bf16, name="wft", tag="wf")
        wf = [wft[:, ct, :] for ct in range(NCT)]
        # residual prefetch tile (used in Phase C)
        xn_all = persist.tile([128, NTILES, DIM], bf16, name="xna", tag="xna")

        # persistent activation tiles: V and attention outputs in fp8, pair
        # (DoubleRow) layout along the contraction dims they feed.
        va_sb = [persist.tile([128, 2, DIM], fp8, name=f"va{b}", tag=f"va{b}") for b in range(BLOC)]
        vt_sb = [persist.tile([128, 2, DIM], fp8, name=f"vt{b}", tag=f"vt{b}") for b in range(BLOC)]
        o_sb = [persist.tile([HD, NH, T], fp8, name=f"o{b}", tag=f"o{b}") for b in range(BLOC)]

        # ========== Phase A+B interleaved: projections + attention ==========
        with tc.tile_pool(name="aacts", bufs=1) as aacts, \
             tc.tile_pool(name="qkp", bufs=3) as qkp, \
             tc.tile_pool(name="ptmp", bufs=6) as ptmp, \
             tc.tile_pool(name="atmp", bufs=3) as atmp, \
             tc.tile_pool(name="artmp", bufs=2) as artmp, \
             tc.tile_pool(name="ppsum", bufs=2, space="PSUM") as ppsum, \
             tc.tile_pool(name="spsum", bufs=2, space="PSUM") as spsum, \
             tc.tile_pool(name="opsum", bufs=1, space="PSUM") as opsum, \
             tc.tile_pool(name="dpsum", bufs=1, space="PSUM") as dpsum:

            # ---- loads (SP queue order == issue order) ----
            def actload(dram, tag, toks):
                t = aacts.tile([128, NCT, toks], fp8, name=tag, tag=tag)
                nc.sync.dma_start(t[:], dram.ap().rearrange("(a p) t -> p a t",
                                                            p=128))
                return t

            def wload(wdram, tag):
                t = aacts.tile([128, NCT, DIM], fp8, name=tag, tag=tag)
                nc.sync.dma_start(t[:], wdram.ap().rearrange("(a p) j -> p a j",
                                                             p=128))
                return t

            sb_ha8 = actload(ha8, "ha8", TOKK)
            sb_wka = wload(wka8, "wka")
            sb_ht8 = actload(ht8, "ht8", TOKK)
            sb_wkt = wload(wkt8, "wkt")
            sb_x8 = aacts.tile([128, NCT, TOKQ], fp8, name="x8", tag="x8")
            for c in range(2):
                nc.sync.dma_start(
                    sb_x8[:, 4 * c:4 * c + 4, :],
                    x8.ap()[4 * c * 128:(4 * c + 4) * 128, :].rearrange(
                        "(a p) t -> p a t", p=128))
            sb_wqa = wload(wqa8, "wqa")
            sb_wqt = wload(wqt8, "wqt")
            sb_wva = wload(wva8, "wva")
            sb_bva = aacts.tile([1, DIM], bf16, name="bva", tag="bva")
            nc.sync.dma_start(sb_bva[:], bva_r.ap())
            sb_wvt = wload(wvt8, "wvt")
            sb_bvt = aacts.tile([1, DIM], bf16, name="bvt", tag="bvt")
            nc.sync.dma_start(sb_bvt[:], bvt_r.ap())
            # phase-C prefetch, behind everything phase A/B needs
            nc.sync.dma_start(wot[:], wo8.ap().rearrange("(a p) j -> p a j",
                                                         p=128))
            nc.sync.dma_start(wft[:], wfT.ap().rearrange("(a p) j -> p a j",
                                                         p=128))
            for c in range(2):
                nc.sync.dma_start(
                    xn_all[:, 4 * c:4 * c + 4, :],
                    xnat.ap()[4 * c * 128:(4 * c + 4) * 128, :].rearrange(
                        "(t p) j -> p t j", p=128))

            # ---- building blocks ----
            def qk_head(j, w, btag, src_sb, tok_len, out_tile, costab, sintab,
                        t1_eng, add_eng):
                """Project + RoPE one head into out_tile ([HD, tok_len])."""
                bias_sb = sb_bcol[btag]
                nchunks = tok_len // 512
                for ch in range(nchunks):
                    sl = slice(ch * 512, (ch + 1) * 512)
                    ps = ppsum.tile([128, 512], f32, tag="proj")
                    for c in range(NPAIR):
                        nc.tensor.matmul(
                            ps[:], w[:, 2 * c:2 * c + 2, j * 128:(j + 1) * 128],
                            src_sb[:, 2 * c:2 * c + 2, sl],
                            start=(c == 0), stop=(c == NPAIR - 1),
                            perf_mode=DR)
                    q1 = ptmp.tile([128, 512], bf16, tag="q1")
                    nc.scalar.activation(q1[:], ps[:], AF.Identity,
                                         bias=bias_sb[:, j:j + 1])
                    if tok_len == T * BLOC and nchunks == BLOC:
                        ctab = costab[:, 0:512]
                        stab = sintab[:, 0:512]
                    else:
                        ctab = costab[:, sl]
                        stab = sintab[:, sl]
                    # rope: pair-swap via DVE stream_shuffle (sign folded
                    # into the sin table); t1/t2/add engines chosen per
                    # stage to balance DVE vs GpSimd.
                    t1 = ptmp.tile([128, 512], bf16, tag="t1")
                    t1_eng.tensor_tensor(t1[:], q1[:], ctab, op=ALU.mult)
                    rot = ptmp.tile([128, 512], bf16, tag="rot")
                    nc.vector.stream_shuffle(rot[:], q1[:], mask=PAIRSWAP)
                    t2 = ptmp.tile([128, 512], bf16, tag="t2")
                    nc.vector.tensor_tensor(t2[:], rot[:], stab, op=ALU.mult)
                    add_eng.tensor_tensor(out_tile[:, sl], t1[:], t2[:],
                                          op=ALU.add)

            def v8(w, src_sb, out_tiles, bias_row):
                for kt_i in range(TOKK // 128):
                    b, kg = divmod(kt_i, 2)
                    for jc in range(2):
                        sl = slice(jc * 512, (jc + 1) * 512)
                        ps = ppsum.tile([128, 512], f32, tag="proj")
                        for c in range(NPAIR):
                            nc.tensor.matmul(
                                ps[:],
                                src_sb[:, 2 * c:2 * c + 2, kt_i * 128:(kt_i + 1) * 128],
                                w[:, 2 * c:2 * c + 2, sl],
                                start=(c == 0), stop=False,
                                perf_mode=DR)
                        # rank-1 bias (ones x bias_row) joins the PSUM group;
                        # the move to SBUF goes on ACT, keeping DVE free.
                        nc.tensor.matmul(ps[:], sb_ones_row[:, 0:128],
                                         bias_row[:, sl], start=False, stop=True)
                        nc.scalar.copy(out_tiles[b][:, kg, sl], ps[:])

            def attention(b, h, qa_t, qt_t, ka_t, kt_t):
                ov = opsum.tile([128, 512], f32, tag="ov")
                qsl = slice(b * T, (b + 1) * T)
                # p tiles in fp8 pair-layout [128 keys, 2 key-groups, 512 q]
                p2 = {}
                for side, (ksb, qsb, esc, kb) in (
                        ("a", (ka_t, qa_t, ESC_A, b * KA)),
                        ("t", (kt_t, qt_t, ESC_T, b * KT))):
                    p2[side] = atmp.tile([128, 2, 512], fp8, tag=f"p2{side}",
                                         name=f"p2{side}")
                    s2 = spsum.tile([128, 2, 512], f32, tag="s2", name="s2")
                    for ci in range(2):
                        nc.tensor.matmul(s2[:, ci, :],
                                         ksb[:, kb + ci * 128:kb + (ci + 1) * 128],
                                         qsb[:, qsl], start=True, stop=True)
                    nc.scalar.activation(p2[side][:], s2[:], AF.Exp, scale=esc)
                # 256-deep DoubleRow: one ov matmul per side, ditto for the
                # denominator, whose ones[128,2,128] stationary broadcasts
                # the key-sum to every partition.
                nc.tensor.matmul(ov[:], va_sb[b][:, :, h * 128:(h + 1) * 128],
                                 p2["a"][:], start=True, stop=False,
                                 skip_group_check=True, perf_mode=DR)
                nc.tensor.matmul(ov[:], vt_sb[b][:, :, h * 128:(h + 1) * 128],
                                 p2["t"][:], start=False, stop=True,
                                 skip_group_check=True, perf_mode=DR)
                denb = dpsum.tile([128, 512], f32, tag="denb")
                nc.tensor.matmul(denb[:], sb_ones8[:], p2["a"][:],
                                 start=True, stop=False, skip_group_check=True,
                                 perf_mode=DR)
                nc.tensor.matmul(denb[:], sb_ones8[:], p2["t"][:],
                                 start=False, stop=True, skip_group_check=True,
                                 perf_mode=DR)
                rb = artmp.tile([128, 512], f32, tag="rb")
                nc.vector.reciprocal_approx_fast(rb[:], denb[:])
                nc.vector.tensor_tensor(o_sb[b][:, h, :], ov[:], rb[:],
                                        op=ALU.mult)

            # ---- schedule: all K heads first (their loads arrive first and
            # the tiles are cheap to keep), then V, then per-head Q + attn.
            ka_all, kt_all = [], []
            for h in range(NH):
                ka_t = qkp.tile([HD, TOKK], bf16, tag="ka", name=f"ka{h}",
                                bufs=NH)
                qk_head(h, sb_wka, "ka", sb_ha8, TOKK, ka_t, sb_cosk, sb_sink,
                        t1_eng=nc.gpsimd, add_eng=nc.vector)
                ka_all.append(ka_t)
            for h in range(NH):
                kt_t = qkp.tile([HD, TOKK], bf16, tag="kt", name=f"kt{h}",
                                bufs=NH)
                qk_head(h, sb_wkt, "kt", sb_ht8, TOKK, kt_t, sb_cosk, sb_sink,
                        t1_eng=nc.vector, add_eng=nc.gpsimd)
                kt_all.append(kt_t)
            v8(sb_wva, sb_ha8, va_sb, sb_bva)
            v8(sb_wvt, sb_ht8, vt_sb, sb_bvt)

            def project_q(h):
                qa_t = qkp.tile([HD, TOKQ], bf16, tag="qa", name=f"qa{h}")
                qk_head(h, sb_wqa, "qa", sb_x8, TOKQ, qa_t, sb_cosq, sb_sinq,
                        t1_eng=nc.gpsimd, add_eng=nc.vector)
                qt_t = qkp.tile([HD, TOKQ], bf16, tag="qt", name=f"qt{h}")
                qk_head(h, sb_wqt, "qt", sb_x8, TOKQ, qt_t, sb_cosq, sb_sinq,
                        t1_eng=nc.gpsimd, add_eng=nc.vector)
                return qa_t, qt_t

            q_prev = project_q(0)
            for h in range(1, NH):
                q_h = project_q(h)
                attention(0, h - 1, q_prev[0], q_prev[1], ka_all[h - 1], kt_all[h - 1])
                attention(1, h - 1, q_prev[0], q_prev[1], ka_all[h - 1], kt_all[h - 1])
                q_prev = q_h
            attention(0, NH - 1, q_prev[0], q_prev[1], ka_all[NH - 1], kt_all[NH - 1])
            attention(1, NH - 1, q_prev[0], q_prev[1], ka_all[NH - 1], kt_all[NH - 1])

        # ================= Phase C: o-proj + LN + FFN =================
        with tc.tile_pool(name="ctmp", bufs=2) as ctmp, \
             tc.tile_pool(name="cres", bufs=3) as cres, \
             tc.tile_pool(name="cps", bufs=2, space="PSUM") as cps:

            def oproj_ln(tt):
                b, t4 = divmod(tt, T // 128)
                x2t = ctmp.tile([128, DIM], f32, tag="x2")
                # fp8 DoubleRow o-proj into x2; the 1/OS descale rides the
                # residual add (x2 = ps/OS + xn) on the DVE.
                for jc in range(2):
                    sl = slice(jc * 512, (jc + 1) * 512)
                    ps = cps.tile([128, 512], f32, tag="op")
                    for c in range(NPAIR):
                        nc.tensor.matmul(
                            ps[:],
                            o_sb[b][:, 2 * c:2 * c + 2, t4 * 128:(t4 + 1) * 128],
                            wot[:, 2 * c:2 * c + 2, sl],
                            start=(c == 0), stop=(c == NPAIR - 1),
                            perf_mode=DR)
                    nc.vector.scalar_tensor_tensor(
                        x2t[:, sl], ps[:], 1.0 / OS, xn_all[:, tt, sl],
                        op0=ALU.mult, op1=ALU.add)
                # LayerNorm
                stats = ctmp.tile([128, 2, 6], f32, tag="stats")
                nc.vector.bn_stats(stats[:, 0, :], x2t[:, 0:512])
                nc.vector.bn_stats(stats[:, 1, :], x2t[:, 512:1024])
                mv = ctmp.tile([128, 2], f32, tag="mv")
                nc.vector.bn_aggr(mv[:], stats[:])
                rstd = ctmp.tile([128, 1], f32, tag="rstd")
                nc.scalar.activation(rstd[:], mv[:, 1:2], AF.Sqrt,
                                     bias=sb_eps[:])
                rstd2 = ctmp.tile([128, 1], f32, tag="rstd2")
                nc.vector.reciprocal(rstd2[:], rstd[:])
                z = ctmp.tile([128, DIM], bf16, tag="z")
                nc.vector.tensor_scalar(z[:], x2t[:],
                                        scalar1=mv[:, 0:1], scalar2=rstd2[:],
                                        op0=ALU.subtract, op1=ALU.mult)
                return z

            def ffn(tt, z, nchunk=2):
                row0 = tt * 128
                # transpose z -> zT (2 halves of 4 blocks each)
                zT = []
                for half in range(2):
                    tp = cps.tile([128, 512], bf16, tag="tp")
                    for q in range(4):
                        cb = half * 4 + q
                        nc.tensor.transpose(
                            tp[:, q * 128:(q + 1) * 128],
                            z[:, cb * 128:(cb + 1) * 128], sb_ident[:])
                    zt = ctmp.tile([128, 512], bf16, tag=f"zT{half}")
                    nc.vector.tensor_copy(zt[:], tp[:])
                    zT.append(zt)
                # FFN + ReLU + store
                csz = DIM // nchunk
                for jc in range(nchunk):
                    sl = slice(jc * csz, (jc + 1) * csz)
                    fp = cps.tile([128, 512], f32, tag="fp")
                    for ct in range(NCT):
                        nc.tensor.matmul(
                            fp[:, 0:csz], zT[ct // 4][:, (ct % 4) * 128:(ct % 4 + 1) * 128],
                            wf[ct][:, sl], start=(ct == 0), stop=False)
                    nc.tensor.matmul(fp[:, 0:csz], sb_ones_row[:], sb_bf[:, sl],
                                     start=False, stop=True)
                    res = cres.tile([128, 512], f32, tag="res")
                    nc.scalar.activation(res[:, 0:csz], fp[:, 0:csz], AF.Relu)
                    nc.sync.dma_start(out_d.ap()[row0:row0 + 128, sl], res[:, 0:csz])

            # software pipeline: o-proj of tile tt+1 is enqueued on the PE
            # ahead of tile tt's transposes+FFN so the LayerNorm chain (DVE)
            # never head-of-line-blocks the PE.
            zprev = oproj_ln(0)
            for tt in range(1, NTILES):
                z = oproj_ln(tt)
                ffn(tt - 1, zprev)
                zprev = z
            ffn(NTILES - 1, zprev, nchunk=4)

    nc.compile()
    return nc


def _to8(a):
    return np.clip(a, -240.0, 240.0).astype(FP8)


def _prep_host(inputs):
    """Host-side preprocessing: expert select, folding, transposes, sharding."""
    x = np.asarray(inputs["x"], dtype=np.float32)
    h_a = np.asarray(inputs["h_a"], dtype=np.float32)
    h_t = np.asarray(inputs["h_t"], dtype=np.float32)
    e = int(np.asarray(inputs["expert_idx"]))
    g = float(1.0 / (1.0 + math.exp(-float(np.asarray(inputs["gating_factor"])[e]))))
    sc = 1.0 / math.sqrt(HD)

    def wT8(w, scale):
        return _to8(np.ascontiguousarray(
            (np.asarray(w, dtype=np.float32)[e] * scale).T))

    def bcol(bv, scale=1.0):
        # [DIM] -> [128, NH]: column h = b[h*128:(h+1)*128]
        return np.ascontiguousarray(
            (np.asarray(bv, dtype=np.float32)[e] * scale).reshape(NH, 128).T
        ).astype(np.float32)

    gamma = np.asarray(inputs["gamma"], dtype=np.float32)[e]
    beta = np.asarray(inputs["beta"], dtype=np.float32)[e]
    w_ffn = np.asarray(inputs["W_ffn"], dtype=np.float32)[e]
    b_ffn = np.asarray(inputs["b_ffn"], dtype=np.float32)[e]
    w_f_eff = w_ffn * gamma[None, :]
    b_f_eff = b_ffn + w_ffn @ beta
    b_o = np.asarray(inputs["b_o"], dtype=np.float32)[e]

    shared = {
        "wqa8": wT8(inputs["W_qa"], sc * QS_A),
        "wqt8": wT8(inputs["W_qt"], sc * g * QS_T),
        "wka8": wT8(inputs["W_ka"], KS),
        "wkt8": wT8(inputs["W_kt"], KS),
        "wva8": wT8(inputs["W_va"], VS),
        "wvt8": wT8(inputs["W_vt"], VS),
        "wo8": wT8(inputs["W_o"], OS / VS),
        "wfT": np.ascontiguousarray(w_f_eff.T).astype(BF16),
        "biascols": np.ascontiguousarray(np.concatenate([
            bcol(inputs["b_qa"], sc * QS_A),
            bcol(inputs["b_qt"], sc * g * QS_T),
            bcol(inputs["b_ka"], KS),
            bcol(inputs["b_kt"], KS),
        ], axis=1)),
        "bva_r": (np.asarray(inputs["b_va"], dtype=np.float32)[e] * VS
                  ).reshape(1, DIM).astype(BF16),
        "bvt_r": (np.asarray(inputs["b_vt"], dtype=np.float32)[e] * VS
                  ).reshape(1, DIM).astype(BF16),
        "bf_row": b_f_eff.reshape(1, DIM).astype(BF16),
    }

    in_maps = []
    for c in range(NCORES):
        xc = x[c * BLOC:(c + 1) * BLOC].reshape(TOKQ, DIM)
        hac = h_a[c * BLOC:(c + 1) * BLOC].reshape(TOKK, DIM)
        htc = h_t[c * BLOC:(c + 1) * BLOC].reshape(TOKK, DIM)
        m = dict(shared)
        m["x8"] = _to8(np.ascontiguousarray(xc.T))
        m["xnat"] = (xc + b_o[None, :]).astype(BF16)
        m["ha8"] = _to8(np.ascontiguousarray(hac.T))
        m["ht8"] = _to8(np.ascontiguousarray(htc.T))
        in_maps.append(m)
    return in_maps


def run(inputs, trace=False):
    from concourse.bass_utils import run_bass_kernel_spmd

    if "nc" not in _CACHE:
        _CACHE["nc"] = build_program()
    nc = _CACHE["nc"]
    in_maps = _prep_host(inputs)
    res = run_bass_kernel_spmd(nc, in_maps, list(range(NCORES)), trace=trace)
    outs = [res.results[c]["out"].reshape(BLOC, T, DIM) for c in range(NCORES)]
    return np.concatenate(outs, axis=0), res


def kernel(**inputs) -> np.ndarray:
    out, _ = run(inputs, trace=False)
    return out
